# revision 17
# baseline (speedup 1.0000x reference)
"""Trainium2 Bass kernel for nn_Encoder_82274393522442.

PointNet-style encoder: 5 pointwise conv (1x1) layers 3->64->128->256->256->1024
with ReLU between, then global max-pool over N=8192 points. B=32, out [32,1024].

Strategy:
- Data-parallel over batch: 8 cores x 4 batches each. No collectives; host concat.
- On-chip layout: channels on partitions, tokens (points) on the free dim.
  Token tile = 512 (one PSUM bank of fp32).
- Matmuls in float32r (fp32 storage, tf32-like multiply): 1 cycle/row on the PE
  (same speed as bf16, ~16x better precision), fp32 PSUM accumulation.
- ReLU+bias fused on ScalarE (ACT) reading PSUM, writing f32r SBUF tiles.
- Max-pool folded in as free-dim tensor_reduce(max) on VectorE straight from
  L5's PSUM, into per-(batch,tile) columns; final small reduce + bias at the end.
"""

import numpy as np

import concourse.bass as bass
import concourse.mybir as mybir
import concourse.tile as tile
from concourse import bacc
from concourse.bass import ts
from concourse.bass_utils import run_bass_kernel_spmd

F32 = mybir.dt.float32
F32R = mybir.dt.float32r
RELU = mybir.ActivationFunctionType.Relu
MAX = mybir.AluOpType.max
AX_X = mybir.AxisListType.X

B, C0, N, Z = 32, 3, 8192, 1024
NCORES = 8
PB = B // NCORES  # batches per core = 4
T = 512  # token tile (one fp32 PSUM bank)
NT = N // T  # 16 token tiles per batch


def build_bass():
    nc = bacc.Bacc("TRN2", target_bir_lowering=False, debug=False, num_devices=NCORES)

    x = nc.dram_tensor("x", [PB, C0, N], F32R, kind="ExternalInput")
    w1t = nc.dram_tensor("w1t", [C0, 64], F32R, kind="ExternalInput")
    w2t = nc.dram_tensor("w2t", [64, 128], F32R, kind="ExternalInput")
    w3t = nc.dram_tensor("w3t", [128, 256], F32R, kind="ExternalInput")
    w4t = nc.dram_tensor("w4t", [128, 2, 256], F32R, kind="ExternalInput")
    w5t = nc.dram_tensor("w5t", [128, 2, 1024], F32R, kind="ExternalInput")
    bias = nc.dram_tensor("bias", [128, 6], F32, kind="ExternalInput")
    b5t = nc.dram_tensor("b5t", [128, 8], F32, kind="ExternalInput")
    out = nc.dram_tensor("out", [PB, Z], F32, kind="ExternalOutput")

    with tile.TileContext(nc) as tc:
        with (
            tc.tile_pool(name="wp", bufs=1) as wp,
            tc.tile_pool(name="xp", bufs=2) as xp,
            tc.tile_pool(name="ap", bufs=3) as ap_,
            tc.tile_pool(name="mp", bufs=2) as mp,
            tc.tile_pool(name="op", bufs=2) as op_,
            tc.tile_pool(name="spp", bufs=2, space="PSUM") as spp,
            tc.tile_pool(name="p5p", bufs=2, space="PSUM") as p5p,
        ):
            tw1 = wp.tile([C0, 64], F32R)
            tw2 = wp.tile([64, 128], F32R)
            tw3 = wp.tile([128, 256], F32R)
            tw4 = wp.tile([128, 2, 256], F32R)
            tw5 = wp.tile([128, 2, 1024], F32R)
            tbias = wp.tile([128, 6], F32)
            tb5 = wp.tile([128, 8], F32)
            # critical-path-first DMA emission: L1-L3 weights + first x chunks,
            # then the big tail weights
            nc.sync.dma_start(tw1, w1t.ap())
            nc.sync.dma_start(tbias, bias.ap())
            nc.sync.dma_start(tw2, w2t.ap())
            nc.sync.dma_start(tw3, w3t.ap())

            NXC = N // 4  # x DMA chunk = 4 token tiles

            def load_x(b):
                xb = xp.tile([C0, N], F32R, tag="xb", name="xb")
                for j in range(4):
                    nc.sync.dma_start(
                        xb[:, ts(j, NXC)], x.ap()[b][:, ts(j, NXC)]
                    )
                return xb

            XB0 = load_x(0)
            nc.sync.dma_start(tw4, w4t.ap())
            nc.sync.dma_start(tb5, b5t.ap())
            nc.sync.dma_start(tw5, w5t.ap())

            # 5-deep software pipeline: iteration i runs L1 of tile i, L2 of
            # tile i-1, L3 of tile i-2, L4 of tile i-3, and the four L5+max
            # chunks of tile i-4 — every relu gets a full iteration of slack
            # before its consumer matmuls, and the DVE reduce for chunk c has
            # several matmul-slots of runway before its p5 slot is needed.
            TILES = PB * NT
            A1, A2, A3, A4 = {}, {}, {}, {}
            XB, MXB = {}, {}

            def emit_chunk(j, c):
                if not (0 <= j < TILES):
                    return
                bp, tp = divmod(j, NT)
                a4p = A4[j]
                mxbp = MXB[bp]
                p5 = p5p.tile([128, 2, T], F32, tag="p5", name="p5")
                for zi in range(2):
                    z = 2 * c + zi
                    for g in range(2):
                        nc.tensor.matmul(
                            p5[:, zi, :],
                            tw5[:, g, ts(z, 128)],
                            a4p[:, g, :],
                            start=(g == 0),
                            stop=(g == 1),
                        )
                nc.vector.tensor_reduce(
                    mxbp[:, tp, 2 * c : 2 * c + 2], p5, axis=AX_X, op=MAX
                )
                if c == 3:
                    del A4[j]
                    if tp == NT - 1:
                        # batch epilogue: max over 16 tile-maxima, add b5, store
                        mxr = op_.tile([128, 8], F32, tag="mxr", name="mxr")
                        nc.vector.tensor_reduce(
                            mxr, mxbp.rearrange("p t z -> p z t"), axis=AX_X, op=MAX
                        )
                        ob = op_.tile([128, 8], F32, tag="ob", name="ob")
                        nc.vector.tensor_add(ob, mxr, tb5)
                        nc.sync.dma_start(
                            out.ap()[bp].rearrange("(z p) -> p z", p=128), ob
                        )

            for i in range(TILES + 4):
                # stage 1: L1 of tile i (3 -> 64)
                if i < TILES:
                    b, t = divmod(i, NT)
                    if t == 0:
                        XB[b] = XB0 if b == 0 else load_x(b)
                        MXB[b] = mp.tile([128, NT, 8], F32, tag="mx", name="mxb")
                    p1 = spp.tile([64, T], F32, tag="sp", name="p1")
                    nc.tensor.matmul(
                        p1, tw1, XB[b][:, ts(t, T)], start=True, stop=True
                    )
                    a1 = ap_.tile([64, T], F32R, tag="a1", name="a1")
                    nc.scalar.activation(a1, p1, RELU, bias=tbias[:64, 0:1])
                    A1[i] = a1
                emit_chunk(i - 4, 0)
                # stage 2: L2 of tile i-1 (64 -> 128)
                if 0 <= i - 1 < TILES:
                    p2 = spp.tile([128, T], F32, tag="sp", name="p2")
                    nc.tensor.matmul(p2, tw2, A1.pop(i - 1), start=True, stop=True)
                    a2 = ap_.tile([128, T], F32R, tag="a2", name="a2")
                    nc.scalar.activation(a2, p2, RELU, bias=tbias[:, 1:2])
                    A2[i - 1] = a2
                emit_chunk(i - 4, 1)
                # stage 3: L3 of tile i-2 (128 -> 256), one 2-bank psum
                if 0 <= i - 2 < TILES:
                    a2p = A2.pop(i - 2)
                    a3 = ap_.tile([128, 2, T], F32R, tag="a3", name="a3")
                    p3 = spp.tile([128, 2, T], F32, tag="sp", name="p3")
                    for g in range(2):
                        nc.tensor.matmul(
                            p3[:, g, :], tw3[:, ts(g, 128)], a2p, start=True, stop=True
                        )
                        nc.scalar.activation(
                            a3[:, g, :], p3[:, g, :], RELU, bias=tbias[:, 2 + g : 3 + g]
                        )
                    A3[i - 2] = a3
                emit_chunk(i - 4, 2)
                # stage 4: L4 of tile i-3 (256 -> 256, accumulate 2 K-halves)
                if 0 <= i - 3 < TILES:
                    a3p = A3.pop(i - 3)
                    a4 = ap_.tile([128, 2, T], F32R, tag="a4", name="a4", bufs=4)
                    p4 = spp.tile([128, 2, T], F32, tag="sp", name="p4")
                    for o in range(2):
                        for g in range(2):
                            nc.tensor.matmul(
                                p4[:, o, :],
                                tw4[:, g, ts(o, 128)],
                                a3p[:, g, :],
                                start=(g == 0),
                                stop=(g == 1),
                            )
                        nc.scalar.activation(
                            a4[:, o, :], p4[:, o, :], RELU, bias=tbias[:, 4 + o : 5 + o]
                        )
                    A4[i - 3] = a4
                emit_chunk(i - 4, 3)

    nc.finalize()
    return nc


_NC_CACHE = None


def _get_nc():
    global _NC_CACHE
    if _NC_CACHE is None:
        _NC_CACHE = build_bass()
    return _NC_CACHE


def _prep_in_maps(inputs):
    f32 = np.float32
    x = np.ascontiguousarray(np.asarray(inputs["x"], dtype=f32))  # [32, 3, 8192]
    W = [np.asarray(inputs[f"W{i}"], dtype=f32) for i in range(1, 6)]
    bvec = [np.asarray(inputs[f"b{i}"], dtype=f32) for i in range(1, 6)]

    w1t = np.ascontiguousarray(W[0].T)  # [3, 64]
    w2t = np.ascontiguousarray(W[1].T)  # [64, 128]
    w3t = np.ascontiguousarray(W[2].T)  # [128, 256]
    # W4.T is [256(in), 256(out)]; -> [in128, g, out] with g the K-half
    w4t = np.ascontiguousarray(W[3].T.reshape(2, 128, 256).transpose(1, 0, 2))
    w5t = np.ascontiguousarray(W[4].T.reshape(2, 128, 1024).transpose(1, 0, 2))

    bias = np.zeros((128, 6), dtype=f32)
    bias[:64, 0] = bvec[0]
    bias[:, 1] = bvec[1]
    bias[:, 2] = bvec[2][:128]
    bias[:, 3] = bvec[2][128:]
    bias[:, 4] = bvec[3][:128]
    bias[:, 5] = bvec[3][128:]
    b5t = np.ascontiguousarray(bvec[4].reshape(8, 128).T)

    shared = {
        "w1t": w1t,
        "w2t": w2t,
        "w3t": w3t,
        "w4t": w4t,
        "w5t": w5t,
        "bias": bias,
        "b5t": b5t,
    }
    in_maps = []
    for c in range(NCORES):
        m = dict(shared)
        m["x"] = x[c * PB : (c + 1) * PB]
        in_maps.append(m)
    return in_maps


def run(inputs, **spmd_kwargs):
    """Run on all 8 cores; returns (output [32,1024] f32, BassKernelResults)."""
    nc = _get_nc()
    in_maps = _prep_in_maps(inputs)
    res = run_bass_kernel_spmd(nc, in_maps, core_ids=list(range(NCORES)), **spmd_kwargs)
    out = np.concatenate([res.results[c]["out"] for c in range(NCORES)], axis=0)
    return out.astype(np.float32), res


def kernel(**inputs):
    out, _ = run(inputs)
    return out


# revision 19
# speedup vs baseline: 1.0073x; 1.0073x over previous
"""Trainium2 Bass kernel for nn_Encoder_82274393522442.

PointNet-style encoder: 5 pointwise conv (1x1) layers 3->64->128->256->256->1024
with ReLU between, then global max-pool over N=8192 points. B=32, out [32,1024].

Strategy:
- Data-parallel over batch: 8 cores x 4 batches each. No collectives; host concat.
- On-chip layout: channels on partitions, tokens (points) on the free dim.
  Token tile = 512 (one PSUM bank of fp32).
- Matmuls in float32r (fp32 storage, tf32-like multiply): 1 cycle/row on the PE
  (same speed as bf16, ~16x better precision), fp32 PSUM accumulation.
- ReLU+bias fused on ScalarE (ACT) reading PSUM, writing f32r SBUF tiles.
- Max-pool folded in as free-dim tensor_reduce(max) on VectorE straight from
  L5's PSUM, into per-(batch,tile) columns; final small reduce + bias at the end.
"""

import numpy as np

import concourse.bass as bass
import concourse.mybir as mybir
import concourse.tile as tile
from concourse import bacc
from concourse.bass import ts
from concourse.bass_utils import run_bass_kernel_spmd

F32 = mybir.dt.float32
F32R = mybir.dt.float32r
RELU = mybir.ActivationFunctionType.Relu
MAX = mybir.AluOpType.max
AX_X = mybir.AxisListType.X

B, C0, N, Z = 32, 3, 8192, 1024
NCORES = 8
PB = B // NCORES  # batches per core = 4
T = 512  # token tile (one fp32 PSUM bank)
NT = N // T  # 16 token tiles per batch


def build_bass():
    nc = bacc.Bacc("TRN2", target_bir_lowering=False, debug=False, num_devices=NCORES)

    x = nc.dram_tensor("x", [PB, C0, N], F32R, kind="ExternalInput")
    w1t = nc.dram_tensor("w1t", [C0, 64], F32R, kind="ExternalInput")
    w2t = nc.dram_tensor("w2t", [64, 128], F32R, kind="ExternalInput")
    w3t = nc.dram_tensor("w3t", [128, 256], F32R, kind="ExternalInput")
    w4t = nc.dram_tensor("w4t", [128, 2, 256], F32R, kind="ExternalInput")
    w5t = nc.dram_tensor("w5t", [128, 2, 1024], F32R, kind="ExternalInput")
    bias = nc.dram_tensor("bias", [128, 6], F32, kind="ExternalInput")
    b5t = nc.dram_tensor("b5t", [128, 8], F32, kind="ExternalInput")
    out = nc.dram_tensor("out", [PB, Z], F32, kind="ExternalOutput")

    with tile.TileContext(nc) as tc:
        with (
            tc.tile_pool(name="wp", bufs=1) as wp,
            tc.tile_pool(name="xp", bufs=2) as xp,
            tc.tile_pool(name="ap", bufs=3) as ap_,
            tc.tile_pool(name="mp", bufs=2) as mp,
            tc.tile_pool(name="op", bufs=2) as op_,
            tc.tile_pool(name="spp", bufs=4, space="PSUM") as spp,
            tc.tile_pool(name="p5p", bufs=2, space="PSUM") as p5p,
        ):
            tw1 = wp.tile([C0, 64], F32R)
            tw2 = wp.tile([64, 128], F32R)
            tw3 = wp.tile([128, 256], F32R)
            tw4 = wp.tile([128, 2, 256], F32R)
            tw5 = wp.tile([128, 2, 1024], F32R)
            tbias = wp.tile([128, 6], F32)
            tb5 = wp.tile([128, 8], F32)
            # critical-path-first DMA emission: L1-L3 weights + first x chunks,
            # then the big tail weights
            nc.sync.dma_start(tw1, w1t.ap())
            nc.sync.dma_start(tbias, bias.ap())
            nc.sync.dma_start(tw2, w2t.ap())
            nc.sync.dma_start(tw3, w3t.ap())

            NXC = N // 4  # x DMA chunk = 4 token tiles

            def load_x(b):
                xb = xp.tile([C0, N], F32R, tag="xb", name="xb")
                for j in range(4):
                    nc.sync.dma_start(
                        xb[:, ts(j, NXC)], x.ap()[b][:, ts(j, NXC)]
                    )
                return xb

            XB0 = load_x(0)
            nc.sync.dma_start(tw4, w4t.ap())
            nc.sync.dma_start(tb5, b5t.ap())
            nc.sync.dma_start(tw5, w5t.ap())

            # 5-deep software pipeline: iteration i runs L1 of tile i, L2 of
            # tile i-1, L3 of tile i-2, L4 of tile i-3, and the four L5+max
            # chunks of tile i-4 — every relu gets a full iteration of slack
            # before its consumer matmuls.
            TILES = PB * NT
            A1, A2, A3, A4 = {}, {}, {}, {}
            XB, MXB = {}, {}

            def emit_chunk(j, c):
                if not (0 <= j < TILES):
                    return
                bp, tp = divmod(j, NT)
                a4p = A4[j]
                mxbp = MXB[bp]
                p5 = p5p.tile([128, 2, T], F32, tag="p5", name="p5")
                for zi in range(2):
                    z = 2 * c + zi
                    for g in range(2):
                        nc.tensor.matmul(
                            p5[:, zi, :],
                            tw5[:, g, ts(z, 128)],
                            a4p[:, g, :],
                            start=(g == 0),
                            stop=(g == 1),
                        )
                nc.vector.tensor_reduce(
                    mxbp[:, tp, 2 * c : 2 * c + 2], p5, axis=AX_X, op=MAX
                )
                if c == 3:
                    del A4[j]
                    if tp == NT - 1:
                        # batch epilogue: max over 16 tile-maxima, add b5, store
                        mxr = op_.tile([128, 8], F32, tag="mxr", name="mxr")
                        nc.vector.tensor_reduce(
                            mxr, mxbp.rearrange("p t z -> p z t"), axis=AX_X, op=MAX
                        )
                        ob = op_.tile([128, 8], F32, tag="ob", name="ob")
                        nc.vector.tensor_add(ob, mxr, tb5)
                        nc.sync.dma_start(
                            out.ap()[bp].rearrange("(z p) -> p z", p=128), ob
                        )

            for i in range(TILES + 4):
                # stage 1: L1 of tile i (3 -> 64)
                if i < TILES:
                    b, t = divmod(i, NT)
                    if t == 0:
                        XB[b] = XB0 if b == 0 else load_x(b)
                        MXB[b] = mp.tile([128, NT, 8], F32, tag="mx", name="mxb")
                    p1 = spp.tile([64, T], F32, tag="sp", name="p1")
                    nc.tensor.matmul(
                        p1, tw1, XB[b][:, ts(t, T)], start=True, stop=True
                    )
                    a1 = ap_.tile([64, T], F32R, tag="a1", name="a1")
                    nc.scalar.activation(a1, p1, RELU, bias=tbias[:64, 0:1])
                    A1[i] = a1
                emit_chunk(i - 4, 0)
                # stage 2: L2 of tile i-1 (64 -> 128)
                if 0 <= i - 1 < TILES:
                    p2 = spp.tile([128, T], F32, tag="sp", name="p2")
                    nc.tensor.matmul(p2, tw2, A1.pop(i - 1), start=True, stop=True)
                    a2 = ap_.tile([128, T], F32R, tag="a2", name="a2")
                    nc.scalar.activation(a2, p2, RELU, bias=tbias[:, 1:2])
                    A2[i - 1] = a2
                emit_chunk(i - 4, 1)
                # stage 3: L3 of tile i-2 (128 -> 256), single-bank psums
                if 0 <= i - 2 < TILES:
                    a2p = A2.pop(i - 2)
                    a3 = ap_.tile([128, 2, T], F32R, tag="a3", name="a3")
                    for g in range(2):
                        p3 = spp.tile([128, T], F32, tag="sp", name=f"p3{g}")
                        nc.tensor.matmul(
                            p3, tw3[:, ts(g, 128)], a2p, start=True, stop=True
                        )
                        nc.scalar.activation(
                            a3[:, g, :], p3, RELU, bias=tbias[:, 2 + g : 3 + g]
                        )
                    A3[i - 2] = a3
                emit_chunk(i - 4, 2)
                # stage 4: L4 of tile i-3 (256 -> 256, accumulate 2 K-halves)
                if 0 <= i - 3 < TILES:
                    a3p = A3.pop(i - 3)
                    a4 = ap_.tile([128, 2, T], F32R, tag="a4", name="a4", bufs=4)
                    for o in range(2):
                        p4 = spp.tile([128, T], F32, tag="sp", name=f"p4{o}")
                        for g in range(2):
                            nc.tensor.matmul(
                                p4,
                                tw4[:, g, ts(o, 128)],
                                a3p[:, g, :],
                                start=(g == 0),
                                stop=(g == 1),
                            )
                        nc.scalar.activation(
                            a4[:, o, :], p4, RELU, bias=tbias[:, 4 + o : 5 + o]
                        )
                    A4[i - 3] = a4
                emit_chunk(i - 4, 3)

    nc.finalize()
    return nc


_NC_CACHE = None


def _get_nc():
    global _NC_CACHE
    if _NC_CACHE is None:
        _NC_CACHE = build_bass()
    return _NC_CACHE


def _prep_in_maps(inputs):
    f32 = np.float32
    x = np.ascontiguousarray(np.asarray(inputs["x"], dtype=f32))  # [32, 3, 8192]
    W = [np.asarray(inputs[f"W{i}"], dtype=f32) for i in range(1, 6)]
    bvec = [np.asarray(inputs[f"b{i}"], dtype=f32) for i in range(1, 6)]

    w1t = np.ascontiguousarray(W[0].T)  # [3, 64]
    w2t = np.ascontiguousarray(W[1].T)  # [64, 128]
    w3t = np.ascontiguousarray(W[2].T)  # [128, 256]
    # W4.T is [256(in), 256(out)]; -> [in128, g, out] with g the K-half
    w4t = np.ascontiguousarray(W[3].T.reshape(2, 128, 256).transpose(1, 0, 2))
    w5t = np.ascontiguousarray(W[4].T.reshape(2, 128, 1024).transpose(1, 0, 2))

    bias = np.zeros((128, 6), dtype=f32)
    bias[:64, 0] = bvec[0]
    bias[:, 1] = bvec[1]
    bias[:, 2] = bvec[2][:128]
    bias[:, 3] = bvec[2][128:]
    bias[:, 4] = bvec[3][:128]
    bias[:, 5] = bvec[3][128:]
    b5t = np.ascontiguousarray(bvec[4].reshape(8, 128).T)

    shared = {
        "w1t": w1t,
        "w2t": w2t,
        "w3t": w3t,
        "w4t": w4t,
        "w5t": w5t,
        "bias": bias,
        "b5t": b5t,
    }
    in_maps = []
    for c in range(NCORES):
        m = dict(shared)
        m["x"] = x[c * PB : (c + 1) * PB]
        in_maps.append(m)
    return in_maps


def run(inputs, **spmd_kwargs):
    """Run on all 8 cores; returns (output [32,1024] f32, BassKernelResults)."""
    nc = _get_nc()
    in_maps = _prep_in_maps(inputs)
    res = run_bass_kernel_spmd(nc, in_maps, core_ids=list(range(NCORES)), **spmd_kwargs)
    out = np.concatenate([res.results[c]["out"] for c in range(NCORES)], axis=0)
    return out.astype(np.float32), res


def kernel(**inputs):
    out, _ = run(inputs)
    return out
